# revision 25
# baseline (speedup 1.0000x reference)
"""LocalOTLoss (masked Sinkhorn OT loss) Trainium2 Bass kernel.

Strategy (8 NeuronCores, pure data parallel over batch):
  - Each core processes BP=64 batches: v[64,256,512], t[64,128,512] f32.
  - Phase 1 (streaming, per batch): DMA v/t, row sumsq (DVE+ACT), rsqrt via
    Sqrt+reciprocal, normalize-and-cast to bf16, PE-transpose the normalized
    bf16 tiles (1 cyc/row), cos-sim matmul A^T = tn @ vn^T in bf16
    (full-speed at any width). Build resident SBUF tensors
    X = exp(A/eps) (bf16, layout [m=NT, b, n=NV+1], dust col = e^g),
    M = (1-A)*X (bf16), and X2 = X^T per batch via hardware DMA transpose
    (bf16 XBAR, [n-chunk, b, c, m]).
  - Phase 2: Sinkhorn in non-log domain: a = mu/(X b), b = nu/(X^T a).
    All-positive sums -> no cancellation. (X b) for all 64 batches via 64
    accumulating bf16 matmuls with block-one-hot stationary (Bdiag stride-66
    trick) -> stacked PSUM [64,256]. (X^T a) via the same trick transposed:
    Adiag stationaries against X2 -> PSUM [64,128]; batched DVE small ops.
    Dustbin handled analytically (constant e^g).
  - Loss = a^T M b per batch -> [64,1] out; host averages 512 values.

Masks are all-ones in this workload (spec fill=ones); a numpy fallback
handles any other mask pattern.
"""

import sys

for _p in ("/opt/trn_rl_repo",):
    if _p not in sys.path:
        sys.path.insert(0, _p)

import numpy as np

import concourse.bass as bass
import concourse.bacc as bacc
import concourse.tile as tile
from concourse import mybir
from concourse.bass_utils import run_bass_kernel_spmd

F32 = mybir.dt.float32
BF16 = mybir.dt.bfloat16
AF = mybir.ActivationFunctionType
ALU = mybir.AluOpType

B, NV, NT, D = 512, 256, 128, 512
NCORES = 8
BP = B // NCORES  # 64 batches per core
EPS = 0.1
ITERS = 5

# effective marginals (mirror reference: exp(log(mu + 1e-9)))
MU_R = 1.0 / (NV + 1e-9) + 1e-9
NU_R = 1.0 / (NT + 1e-9) + 1e-9
DUST = 1.0 + 1e-9


def build_bass(eg: float, bp: int = BP, stage: str = "full") -> bass.Bass:
    """Build the per-core Bass module. eg = exp(gamma/eps)."""
    nc = bacc.Bacc(trn_type="TRN2")
    v = nc.dram_tensor("v", [bp, NV, D], F32, kind="ExternalInput")
    t = nc.dram_tensor("t", [bp, NT, D], F32, kind="ExternalInput")
    # invs[b, p, :] = (1/|v_p|, 1/|v_{p+128}|, 1/|t_p|) host-precomputed
    invs = nc.dram_tensor("invs", [bp, 128, 3], F32, kind="ExternalInput")
    out = nc.dram_tensor("out", [bp, 1], F32, kind="ExternalOutput")
    ident_np = np.eye(128, dtype=np.float32)
    ident_dram = nc.inline_tensor(ident_np, name="ident")

    with tile.TileContext(nc) as tc:
        _body(nc, tc, v, t, invs, out, ident_dram, eg, bp, stage)
    nc.finalize()
    return nc


def _phase1_batch(nc, pools, b, v, t, ident_bf, invs_sb, X_all, M_all, X2,
                  stage="full"):
    io, work, pa = pools
    vt = io.tile([128, 2, D], F32, tag="vt")
    nc.sync.dma_start(out=vt, in_=v[b].rearrange("(h p) d -> p h d", p=128))
    tt = io.tile([128, D], F32, tag="tt")
    nc.sync.dma_start(out=tt, in_=t[b])

    # --- normalize + cast to bf16: v rows on DVE, t rows on ACT ---
    vb = work.tile([128, 2, D], BF16, tag="vb")
    nc.vector.tensor_scalar(
        out=vb[:, 0, :], in0=vt[:, 0, :], scalar1=invs_sb[:, b, 0:1],
        scalar2=None, op0=ALU.mult,
    )
    nc.vector.tensor_scalar(
        out=vb[:, 1, :], in0=vt[:, 1, :], scalar1=invs_sb[:, b, 1:2],
        scalar2=None, op0=ALU.mult,
    )
    tb = work.tile([128, D], BF16, tag="tb")
    nc.scalar.activation(out=tb, in_=tt, func=AF.Copy, scale=invs_sb[:, b, 2:3])
    if stage == "s2":
        nc.vector.tensor_copy(out=X_all[:, b, 0:NV], in_=vb[:, 0, 0:NV])
        return

    # --- transposes via DMA XBAR (bf16 SBUF->SBUF), no PE involvement ---
    # vTb[p, c, n] = v_norm[n, 128c+p]; tTb[p, c, m] = t_norm[m, 128c+p]
    vTb = work.tile([128, 4, 256], BF16, tag="vTb")
    nc.sync.dma_start(out=vTb[:, :, 0:128], in_=vb[:, 0, :], transpose=True)
    nc.scalar.dma_start(out=vTb[:, :, 128:256], in_=vb[:, 1, :], transpose=True)
    tTb = work.tile([128, 4, 128], BF16, tag="tTb")
    nc.scalar.dma_start(out=tTb, in_=tb, transpose=True)
    if stage == "s3":
        nc.vector.tensor_copy(out=X_all[:, b, 0:NV], in_=vTb[:, 0, :])
        return

    # --- cos-sim matmul: psA[m=nt, n=nv] = sum_d t_n[m,d] v_n[n,d] ---
    psA = pa.tile([128, 256], F32, tag="psA")
    for c in range(4):
        nc.tensor.matmul(
            psA,
            lhsT=tTb[:, c, :],
            rhs=vTb[:, c, :],
            start=(c == 0),
            stop=(c == 3),
        )

    # --- X = exp(A/eps) (ACT), om = 1-A (ACT), M = om*X (Pool) ---
    nc.scalar.activation(
        out=X_all[:, b, 0:NV], in_=psA, func=AF.Exp, scale=1.0 / EPS
    )
    if stage == "s4":
        return
    om = work.tile([128, 256], BF16, tag="om")
    nc.scalar.activation(out=om, in_=psA, func=AF.Copy, scale=-1.0, bias=1.0)
    nc.gpsimd.tensor_mul(out=M_all[:, b, :], in0=om, in1=X_all[:, b, 0:NV])

    # --- X2 = X^T via hardware DMA transpose (bf16 XBAR) ---
    if stage != "p1nx":
        nc.scalar.dma_start(
            out=X2[:, b, :, :], in_=X_all[:, b, 0:NV], transpose=True
        )


def _body(nc, tc, v, t, invs, out, ident_dram, eg, bp, stage="full"):
    from contextlib import ExitStack

    with ExitStack() as ctx:
        consts = ctx.enter_context(tc.tile_pool(name="consts", bufs=1))
        big = ctx.enter_context(tc.tile_pool(name="big", bufs=1))

        ident_sb = consts.tile([128, 128], F32)
        nc.sync.dma_start(out=ident_sb, in_=ident_dram[:, :])
        ident_bf = consts.tile([128, 128], BF16)
        nc.vector.tensor_copy(out=ident_bf, in_=ident_sb)
        invs_sb = consts.tile([128, bp, 3], F32)
        nc.sync.dma_start(out=invs_sb, in_=invs.rearrange("b p c -> p b c"))

        # Resident: X_all[m, b, n] bf16 (col NV = e^g), M_all, X2[n%128, b, c, m]
        X_all = big.tile([128, bp, NV + 1], BF16)
        M_all = big.tile([128, bp, NV], BF16)
        X2 = big.tile([128, bp, 2, NT], BF16)
        nc.vector.memset(X_all[:, :, NV : NV + 1], eg)
        if stage != "full":
            # bisection stubs leave tensors unwritten; placate the allocator
            nc.vector.memset(X_all[:, :, 0:NV], 1.0)
            nc.vector.memset(M_all, 1.0)
            nc.vector.memset(X2, 1.0)

        # ---------------- Phase 1: build X, M, X2 ----------------
        with ExitStack() as p1:
            io = p1.enter_context(tc.tile_pool(name="io", bufs=3))
            work = p1.enter_context(tc.tile_pool(name="work", bufs=3))
            pa = p1.enter_context(tc.tile_pool(name="pa", bufs=2, space="PSUM"))
            pools = (io, work, pa)
            for b in range(bp):
                _phase1_batch(nc, pools, b, v, t, ident_bf, invs_sb,
                              X_all, M_all, X2, stage)

        if stage != "full":
            with tc.tile_pool(name="dbg", bufs=1) as dbg:
                lossc = dbg.tile([bp, 1], F32)
                nc.vector.tensor_reduce(
                    out=lossc,
                    in_=X_all[0:bp, 0, :],
                    axis=mybir.AxisListType.X,
                    op=ALU.add,
                )
                nc.sync.dma_start(out=out[:, :], in_=lossc)
            return

        # ---------------- Phase 2: Sinkhorn iterations ----------------
        with ExitStack() as p2:
            ph2 = p2.enter_context(tc.tile_pool(name="ph2", bufs=1))
            p2w = p2.enter_context(tc.tile_pool(name="p2w", bufs=2))
            pps = p2.enter_context(tc.tile_pool(name="pps", bufs=2, space="PSUM"))
            ppt = p2.enter_context(tc.tile_pool(name="ppt", bufs=2, space="PSUM"))

            # diagonal-slot stationaries: batch b's one-hot col at abs col 66*b
            Bdiag = ph2.tile([128, 65 * bp], BF16)
            nc.vector.memset(Bdiag, 0.0)
            Adiag0 = ph2.tile([128, 65 * bp], BF16)
            nc.vector.memset(Adiag0, 0.0)
            Adiag1 = ph2.tile([128, 65 * bp], BF16)
            nc.vector.memset(Adiag1, 0.0)

            def slots(tile_):
                return bass.AP(
                    tensor=tile_.tensor,
                    offset=tile_.offset,
                    ap=[list(tile_.ap[0]), [66, bp]],
                )

            BmatT0 = ph2.tile([128, bp], BF16)
            nc.vector.memset(BmatT0, 1.0)
            bmat_rm = ph2.tile([bp, NT], BF16)  # b row-major [b, m]
            nc.vector.memset(bmat_rm, 1.0)
            bdust = ph2.tile([bp, 1], F32)
            nc.vector.memset(bdust, 1.0)
            Amat = ph2.tile([bp, NV + 1], BF16)
            adust = ph2.tile([bp, 1], F32)
            lossc = ph2.tile([bp, 1], F32)

            psBT = None
            for it in range(ITERS):
                # -- u-update: a = mu / (X b + eg*bdust) --
                nc.vector.tensor_copy(
                    out=slots(Bdiag), in_=BmatT0 if it == 0 else psBT
                )
                psS = pps.tile([bp, 256], F32, tag="psS")
                for b in range(bp):
                    nc.tensor.matmul(
                        psS,
                        lhsT=Bdiag[:, 65 * b : 65 * b + bp],
                        rhs=X_all[:, b, 0:NV],
                        start=(b == 0),
                        stop=(b == bp - 1),
                    )
                bd_eg = p2w.tile([bp, 1], F32, tag="bd_eg")
                nc.vector.tensor_scalar_mul(bd_eg, bdust, eg)
                den = p2w.tile([bp, 256], F32, tag="den")
                nc.vector.tensor_scalar(
                    out=den, in0=psS, scalar1=bd_eg, scalar2=None, op0=ALU.add
                )
                rec = p2w.tile([bp, 256], F32, tag="rec")
                nc.vector.reciprocal(out=rec, in_=den)
                nc.vector.tensor_scalar_mul(Amat[:, 0:NV], rec, MU_R)
                # a_dust = DUST / (eg * (sum_m b + bdust))
                psum_b = p2w.tile([bp, 1], F32, tag="psum_b")
                nc.vector.tensor_reduce(
                    out=psum_b, in_=bmat_rm, axis=mybir.AxisListType.X, op=ALU.add
                )
                sbt = p2w.tile([bp, 1], F32, tag="sbt")
                nc.vector.tensor_add(out=sbt, in0=psum_b, in1=bdust)
                sbt2 = p2w.tile([bp, 1], F32, tag="sbt2")
                nc.vector.tensor_scalar_mul(sbt2, sbt, eg)
                sbt3 = p2w.tile([bp, 1], F32, tag="sbt3")
                nc.vector.reciprocal(out=sbt3, in_=sbt2)
                nc.vector.tensor_scalar_mul(adust, sbt3, DUST)
                nc.vector.tensor_copy(out=Amat[:, NV : NV + 1], in_=adust)

                # -- w-update: b = nu / (X^T a + eg*adust) --
                psAT = ppt.tile([128, 2, bp], BF16, tag="psAT")
                nc.tensor.transpose(
                    out=psAT[:, 0, :], in_=Amat[:, 0:128], identity=ident_bf[0:bp, 0:bp]
                )
                nc.tensor.transpose(
                    out=psAT[:, 1, :], in_=Amat[:, 128:256], identity=ident_bf[0:bp, 0:bp]
                )
                nc.vector.tensor_copy(out=slots(Adiag0), in_=psAT[:, 0, :])
                nc.vector.tensor_copy(out=slots(Adiag1), in_=psAT[:, 1, :])
                psT = ppt.tile([bp, NT], F32, tag="psT")
                for b in range(bp):
                    nc.tensor.matmul(
                        psT,
                        lhsT=Adiag0[:, 65 * b : 65 * b + bp],
                        rhs=X2[:, b, 0, :],
                        start=(b == 0),
                        stop=False,
                    )
                    nc.tensor.matmul(
                        psT,
                        lhsT=Adiag1[:, 65 * b : 65 * b + bp],
                        rhs=X2[:, b, 1, :],
                        start=False,
                        stop=(b == bp - 1),
                    )
                ad_eg = p2w.tile([bp, 1], F32, tag="ad_eg")
                nc.vector.tensor_scalar_mul(ad_eg, adust, eg)
                denw = p2w.tile([bp, NT], F32, tag="denw")
                nc.vector.tensor_scalar(
                    out=denw, in0=psT, scalar1=ad_eg, scalar2=None, op0=ALU.add
                )
                recw = p2w.tile([bp, NT], F32, tag="recw")
                nc.vector.reciprocal(out=recw, in_=denw)
                nc.vector.tensor_scalar_mul(bmat_rm, recw, NU_R)
                # b_dust = DUST / (eg * sum_n a_total)
                sa = p2w.tile([bp, 1], F32, tag="sa")
                nc.vector.tensor_reduce(
                    out=sa, in_=Amat, axis=mybir.AxisListType.X, op=ALU.add
                )
                sa2 = p2w.tile([bp, 1], F32, tag="sa2")
                nc.vector.tensor_scalar_mul(sa2, sa, eg)
                sa3 = p2w.tile([bp, 1], F32, tag="sa3")
                nc.vector.reciprocal(out=sa3, in_=sa2)
                nc.vector.tensor_scalar_mul(bdust, sa3, DUST)
                psBT = ppt.tile([128, bp], BF16, tag="psBT")
                nc.tensor.transpose(
                    out=psBT, in_=bmat_rm, identity=ident_bf[0:bp, 0:bp]
                )

            # -- loss = a^T M b per batch --
            nc.vector.tensor_copy(out=slots(Bdiag), in_=psBT)
            psL = pps.tile([bp, 256], F32, tag="psS")
            for b in range(bp):
                nc.tensor.matmul(
                    psL,
                    lhsT=Bdiag[:, 65 * b : 65 * b + bp],
                    rhs=M_all[:, b, :],
                    start=(b == 0),
                    stop=(b == bp - 1),
                )
            ltmp = p2w.tile([bp, 256], F32, tag="ltmp")
            nc.vector.tensor_mul(out=ltmp, in0=psL, in1=Amat[:, 0:NV])
            nc.vector.tensor_reduce(
                out=lossc, in_=ltmp, axis=mybir.AxisListType.X, op=ALU.add
            )
            nc.sync.dma_start(out=out[:, :], in_=lossc)


_nc_cache: dict = {}


def _numpy_fallback(v, t, v_mask, t_mask, gamma):
    """Exact numpy port of the reference (for non-all-ones masks)."""
    NEG_INF = -1e6
    v = v.astype(np.float32)
    t = t.astype(np.float32)
    vn = v / np.maximum(np.sqrt((v * v).sum(-1, keepdims=True)), 1e-12)
    tn = t / np.maximum(np.sqrt((t * t).sum(-1, keepdims=True)), 1e-12)
    A = np.einsum("bnd,bmd->bnm", vn, tn).astype(np.float32)
    A_raw = A.copy()
    A = np.where(v_mask[:, :, None], A, NEG_INF)
    A = np.where(t_mask[:, None, :], A, NEG_INF)
    Bn = A.shape[0]
    g = np.float32(gamma)
    A_aug = np.concatenate([A, np.full((Bn, NV, 1), g, np.float32)], axis=2)
    A_aug = np.concatenate(
        [A_aug, np.full((Bn, 1, NT + 1), g, np.float32)], axis=1
    )
    v_counts = v_mask.sum(1, keepdims=True) + 1e-9
    mu_real = v_mask.astype(np.float32) / v_counts
    t_counts = t_mask.sum(1, keepdims=True) + 1e-9
    nu_real = t_mask.astype(np.float32) / t_counts
    ones = np.ones((Bn, 1), np.float32)
    mu = np.concatenate([mu_real, ones], 1)
    nu = np.concatenate([nu_real, ones], 1)
    K = A_aug / EPS
    log_mu = np.log(mu + 1e-9)
    log_nu = np.log(nu + 1e-9)
    u = np.zeros_like(mu)
    w = np.zeros_like(nu)

    def lse(x, axis):
        m = x.max(axis=axis, keepdims=True)
        return (m + np.log(np.exp(x - m).sum(axis=axis, keepdims=True))).squeeze(axis)

    for _ in range(ITERS):
        u = log_mu - lse(K + w[:, None, :], 2)
        w = log_nu - lse(K + u[:, :, None], 1)
    T = np.exp(u[:, :, None] + w[:, None, :] + K)
    loss = (T[:, :NV, :NT] * (1.0 - A_raw)).sum((1, 2))
    return np.float32(loss.mean())


import os


def kernel(v, t, v_mask, t_mask, gamma):
    v = np.ascontiguousarray(np.asarray(v), dtype=np.float32)
    t = np.ascontiguousarray(np.asarray(t), dtype=np.float32)
    v_mask = np.asarray(v_mask)
    t_mask = np.asarray(t_mask)
    gamma_f = float(np.asarray(gamma))

    if not (v_mask.all() and t_mask.all()):
        return _numpy_fallback(v, t, v_mask, t_mask, gamma_f)

    try:
        eg = float(np.exp(np.float32(gamma_f) / np.float32(EPS)))
        key = (eg, v.shape, t.shape)
        if key not in _nc_cache:
            _nc_cache[key] = build_bass(eg)
        nc = _nc_cache[key]

        # host-side row norms (reference-exact f32 semantics)
        inv_v = 1.0 / np.maximum(
            np.sqrt((v * v).sum(-1)), np.float32(1e-12)
        ).astype(np.float32)
        inv_t = 1.0 / np.maximum(
            np.sqrt((t * t).sum(-1)), np.float32(1e-12)
        ).astype(np.float32)
        invs = np.empty((B, 128, 3), np.float32)
        invs[:, :, 0] = inv_v[:, 0:128]
        invs[:, :, 1] = inv_v[:, 128:256]
        invs[:, :, 2] = inv_t
        invs = np.ascontiguousarray(invs)

        in_maps = [
            {
                "v": v[i * BP : (i + 1) * BP],
                "t": t[i * BP : (i + 1) * BP],
                "invs": invs[i * BP : (i + 1) * BP],
            }
            for i in range(NCORES)
        ]
        res = run_bass_kernel_spmd(nc, in_maps, core_ids=list(range(NCORES)))
        losses = np.concatenate([r["out"][:, 0] for r in res.results])
        return np.float32(np.mean(losses.astype(np.float64)))
    except Exception:
        if os.environ.get("KERNEL_NO_FALLBACK"):
            raise
        return _numpy_fallback(v, t, v_mask, t_mask, gamma_f)


if __name__ == "__main__":
    rng = np.random.default_rng(0)
    v = rng.standard_normal((B, NV, D), dtype=np.float32)
    t = rng.standard_normal((B, NT, D), dtype=np.float32)
    vm = np.ones((B, NV), bool)
    tm = np.ones((B, NT), bool)
    print(kernel(v, t, vm, tm, np.float32(0.1)))


# revision 26
# speedup vs baseline: 1.2319x; 1.2319x over previous
"""LocalOTLoss (masked Sinkhorn OT loss) Trainium2 Bass kernel.

Strategy (8 NeuronCores, pure data parallel over batch):
  - Each core processes BP=64 batches: v[64,256,512], t[64,128,512] f32.
  - Phase 1 (streaming, per batch): DMA v/t, row sumsq (DVE+ACT), rsqrt via
    Sqrt+reciprocal, normalize-and-cast to bf16, PE-transpose the normalized
    bf16 tiles (1 cyc/row), cos-sim matmul A^T = tn @ vn^T in bf16
    (full-speed at any width). Build resident SBUF tensors
    X = exp(A/eps) (bf16, layout [m=NT, b, n=NV+1], dust col = e^g),
    M = (1-A)*X (bf16), and X2 = X^T per batch via hardware DMA transpose
    (bf16 XBAR, [n-chunk, b, c, m]).
  - Phase 2: Sinkhorn in non-log domain: a = mu/(X b), b = nu/(X^T a).
    All-positive sums -> no cancellation. (X b) for all 64 batches via 64
    accumulating bf16 matmuls with block-one-hot stationary (Bdiag stride-66
    trick) -> stacked PSUM [64,256]. (X^T a) via the same trick transposed:
    Adiag stationaries against X2 -> PSUM [64,128]; batched DVE small ops.
    Dustbin handled analytically (constant e^g).
  - Loss = a^T M b per batch -> [64,1] out; host averages 512 values.

Masks are all-ones in this workload (spec fill=ones); a numpy fallback
handles any other mask pattern.
"""

import sys

for _p in ("/opt/trn_rl_repo",):
    if _p not in sys.path:
        sys.path.insert(0, _p)

import numpy as np

import concourse.bass as bass
import concourse.bacc as bacc
import concourse.tile as tile
from concourse import mybir
from concourse.bass_utils import run_bass_kernel_spmd

F32 = mybir.dt.float32
BF16 = mybir.dt.bfloat16
AF = mybir.ActivationFunctionType
ALU = mybir.AluOpType

B, NV, NT, D = 512, 256, 128, 512
NCORES = 8
BP = B // NCORES  # 64 batches per core
EPS = 0.1
ITERS = 5

# effective marginals (mirror reference: exp(log(mu + 1e-9)))
MU_R = 1.0 / (NV + 1e-9) + 1e-9
NU_R = 1.0 / (NT + 1e-9) + 1e-9
DUST = 1.0 + 1e-9


def build_bass(eg: float, bp: int = BP, stage: str = "full") -> bass.Bass:
    """Build the per-core Bass module. eg = exp(gamma/eps)."""
    nc = bacc.Bacc(trn_type="TRN2")
    v = nc.dram_tensor("v", [bp, NV, D], F32, kind="ExternalInput")
    t = nc.dram_tensor("t", [bp, NT, D], F32, kind="ExternalInput")
    # invs[b, p, :] = (1/|v_p|, 1/|v_{p+128}|, 1/|t_p|) host-precomputed
    invs = nc.dram_tensor("invs", [bp, 128, 3], F32, kind="ExternalInput")
    out = nc.dram_tensor("out", [bp, 1], F32, kind="ExternalOutput")
    ident_np = np.eye(128, dtype=np.float32)
    ident_dram = nc.inline_tensor(ident_np, name="ident")

    with tile.TileContext(nc) as tc:
        _body(nc, tc, v, t, invs, out, ident_dram, eg, bp, stage)
    nc.finalize()
    return nc


def _phase1_batch(nc, pools, b, v, t, ident_bf, invs_sb, X_all, M_all, X2,
                  stage="full"):
    io, work, pa = pools
    vt = io.tile([128, 2, D], F32, tag="vt")
    nc.gpsimd.dma_start(out=vt, in_=v[b].rearrange("(h p) d -> p h d", p=128))
    tt = io.tile([128, D], F32, tag="tt")
    nc.gpsimd.dma_start(out=tt, in_=t[b])

    # --- normalize + cast to bf16: v rows on DVE, t rows on ACT ---
    vb = work.tile([128, 2, D], BF16, tag="vb")
    nc.vector.tensor_scalar(
        out=vb[:, 0, :], in0=vt[:, 0, :], scalar1=invs_sb[:, b, 0:1],
        scalar2=None, op0=ALU.mult,
    )
    nc.vector.tensor_scalar(
        out=vb[:, 1, :], in0=vt[:, 1, :], scalar1=invs_sb[:, b, 1:2],
        scalar2=None, op0=ALU.mult,
    )
    tb = work.tile([128, D], BF16, tag="tb")
    nc.scalar.activation(out=tb, in_=tt, func=AF.Copy, scale=invs_sb[:, b, 2:3])
    if stage == "s2":
        nc.vector.tensor_copy(out=X_all[:, b, 0:NV], in_=vb[:, 0, 0:NV])
        return

    # --- transposes via DMA XBAR (bf16 SBUF->SBUF), no PE involvement ---
    # vT8[p, c', k]: c'<4 -> v_norm[k, 128c'+p], c'>=4 -> v_norm[128+k, ...]
    vT8 = work.tile([128, 8, 128], BF16, tag="vT8")
    nc.sync.dma_start(
        out=vT8, in_=vb.rearrange("p h d -> p (h d)"), transpose=True
    )
    tTb = work.tile([128, 4, 128], BF16, tag="tTb")
    nc.scalar.dma_start(out=tTb, in_=tb, transpose=True)
    if stage == "s3":
        nc.vector.tensor_copy(out=X_all[:, b, 0:NV], in_=vT8[:, 0:2, :])
        return

    # --- cos-sim matmul: psA[m=nt, n=nv] = sum_d t_n[m,d] v_n[n,d] ---
    vview = vT8.rearrange("p (h c) k -> p c h k", h=2)
    psA = pa.tile([128, 256], F32, tag="psA")
    for c in range(4):
        nc.tensor.matmul(
            psA,
            lhsT=tTb[:, c, :],
            rhs=vview[:, c, :, :],
            start=(c == 0),
            stop=(c == 3),
        )

    # --- X = exp(A/eps) (ACT), om = 1-A (ACT), M = om*X (Pool) ---
    nc.scalar.activation(
        out=X_all[:, b, 0:NV], in_=psA, func=AF.Exp, scale=1.0 / EPS
    )
    if stage == "s4":
        return
    om = work.tile([128, 256], BF16, tag="om")
    nc.scalar.activation(out=om, in_=psA, func=AF.Copy, scale=-1.0, bias=1.0)
    nc.gpsimd.tensor_mul(out=M_all[:, b, :], in0=om, in1=X_all[:, b, 0:NV])

    # --- X2 = X^T via hardware DMA transpose (bf16 XBAR) ---
    if stage != "p1nx":
        nc.scalar.dma_start(
            out=X2[:, b, :, :], in_=X_all[:, b, 0:NV], transpose=True
        )


def _body(nc, tc, v, t, invs, out, ident_dram, eg, bp, stage="full"):
    from contextlib import ExitStack

    with ExitStack() as ctx:
        consts = ctx.enter_context(tc.tile_pool(name="consts", bufs=1))
        big = ctx.enter_context(tc.tile_pool(name="big", bufs=1))

        ident_sb = consts.tile([128, 128], F32)
        nc.sync.dma_start(out=ident_sb, in_=ident_dram[:, :])
        ident_bf = consts.tile([128, 128], BF16)
        nc.vector.tensor_copy(out=ident_bf, in_=ident_sb)
        invs_sb = consts.tile([128, bp, 3], F32)
        nc.sync.dma_start(out=invs_sb, in_=invs.rearrange("b p c -> p b c"))

        # Resident: X_all[m, b, n] bf16 (col NV = e^g), M_all, X2[n%128, b, c, m]
        X_all = big.tile([128, bp, NV + 1], BF16)
        M_all = big.tile([128, bp, NV], BF16)
        X2 = big.tile([128, bp, 2, NT], BF16)
        nc.vector.memset(X_all[:, :, NV : NV + 1], eg)
        if stage != "full":
            # bisection stubs leave tensors unwritten; placate the allocator
            nc.vector.memset(X_all[:, :, 0:NV], 1.0)
            nc.vector.memset(M_all, 1.0)
            nc.vector.memset(X2, 1.0)

        # ---------------- Phase 1: build X, M, X2 ----------------
        with ExitStack() as p1:
            io = p1.enter_context(tc.tile_pool(name="io", bufs=3))
            work = p1.enter_context(tc.tile_pool(name="work", bufs=3))
            pa = p1.enter_context(tc.tile_pool(name="pa", bufs=2, space="PSUM"))
            pools = (io, work, pa)
            for b in range(bp):
                _phase1_batch(nc, pools, b, v, t, ident_bf, invs_sb,
                              X_all, M_all, X2, stage)

        if stage != "full":
            with tc.tile_pool(name="dbg", bufs=1) as dbg:
                lossc = dbg.tile([bp, 1], F32)
                nc.vector.tensor_reduce(
                    out=lossc,
                    in_=X_all[0:bp, 0, :],
                    axis=mybir.AxisListType.X,
                    op=ALU.add,
                )
                nc.sync.dma_start(out=out[:, :], in_=lossc)
            return

        # ---------------- Phase 2: Sinkhorn iterations ----------------
        with ExitStack() as p2:
            ph2 = p2.enter_context(tc.tile_pool(name="ph2", bufs=1))
            p2w = p2.enter_context(tc.tile_pool(name="p2w", bufs=2))
            pps = p2.enter_context(tc.tile_pool(name="pps", bufs=2, space="PSUM"))
            ppt = p2.enter_context(tc.tile_pool(name="ppt", bufs=2, space="PSUM"))

            # diagonal-slot stationaries: batch b's one-hot col at abs col 66*b
            Bdiag = ph2.tile([128, 65 * bp], BF16)
            nc.vector.memset(Bdiag, 0.0)
            Adiag0 = ph2.tile([128, 65 * bp], BF16)
            nc.vector.memset(Adiag0, 0.0)
            Adiag1 = ph2.tile([128, 65 * bp], BF16)
            nc.vector.memset(Adiag1, 0.0)

            def slots(tile_):
                return bass.AP(
                    tensor=tile_.tensor,
                    offset=tile_.offset,
                    ap=[list(tile_.ap[0]), [66, bp]],
                )

            BmatT0 = ph2.tile([128, bp], BF16)
            nc.vector.memset(BmatT0, 1.0)
            bmat_rm = ph2.tile([bp, NT], BF16)  # b row-major [b, m]
            nc.vector.memset(bmat_rm, 1.0)
            bdust = ph2.tile([bp, 1], F32)
            nc.vector.memset(bdust, 1.0)
            Amat = ph2.tile([bp, NV + 1], BF16)
            adust = ph2.tile([bp, 1], F32)
            lossc = ph2.tile([bp, 1], F32)

            psBT = None
            for it in range(ITERS):
                # -- u-update: a = mu / (X b + eg*bdust) --
                nc.vector.tensor_copy(
                    out=slots(Bdiag), in_=BmatT0 if it == 0 else psBT
                )
                psS = pps.tile([bp, 256], F32, tag="psS")
                for b in range(bp):
                    nc.tensor.matmul(
                        psS,
                        lhsT=Bdiag[:, 65 * b : 65 * b + bp],
                        rhs=X_all[:, b, 0:NV],
                        start=(b == 0),
                        stop=(b == bp - 1),
                    )
                bd_eg = p2w.tile([bp, 1], F32, tag="bd_eg")
                nc.vector.tensor_scalar_mul(bd_eg, bdust, eg)
                den = p2w.tile([bp, 256], F32, tag="den")
                nc.vector.tensor_scalar(
                    out=den, in0=psS, scalar1=bd_eg, scalar2=None, op0=ALU.add
                )
                rec = p2w.tile([bp, 256], F32, tag="rec")
                nc.vector.reciprocal(out=rec, in_=den)
                nc.vector.tensor_scalar_mul(Amat[:, 0:NV], rec, MU_R)
                # a_dust = DUST / (eg * (sum_m b + bdust))
                psum_b = p2w.tile([bp, 1], F32, tag="psum_b")
                nc.vector.tensor_reduce(
                    out=psum_b, in_=bmat_rm, axis=mybir.AxisListType.X, op=ALU.add
                )
                sbt = p2w.tile([bp, 1], F32, tag="sbt")
                nc.vector.tensor_add(out=sbt, in0=psum_b, in1=bdust)
                sbt2 = p2w.tile([bp, 1], F32, tag="sbt2")
                nc.vector.tensor_scalar_mul(sbt2, sbt, eg)
                sbt3 = p2w.tile([bp, 1], F32, tag="sbt3")
                nc.vector.reciprocal(out=sbt3, in_=sbt2)
                nc.vector.tensor_scalar_mul(adust, sbt3, DUST)
                nc.vector.tensor_copy(out=Amat[:, NV : NV + 1], in_=adust)

                # -- w-update: b = nu / (X^T a + eg*adust) --
                psAT = ppt.tile([128, 2, bp], BF16, tag="psAT")
                nc.tensor.transpose(
                    out=psAT[:, 0, :], in_=Amat[:, 0:128], identity=ident_bf[0:bp, 0:bp]
                )
                nc.tensor.transpose(
                    out=psAT[:, 1, :], in_=Amat[:, 128:256], identity=ident_bf[0:bp, 0:bp]
                )
                nc.vector.tensor_copy(out=slots(Adiag0), in_=psAT[:, 0, :])
                nc.vector.tensor_copy(out=slots(Adiag1), in_=psAT[:, 1, :])
                psT = ppt.tile([bp, NT], F32, tag="psT")
                for b in range(bp):
                    nc.tensor.matmul(
                        psT,
                        lhsT=Adiag0[:, 65 * b : 65 * b + bp],
                        rhs=X2[:, b, 0, :],
                        start=(b == 0),
                        stop=False,
                    )
                    nc.tensor.matmul(
                        psT,
                        lhsT=Adiag1[:, 65 * b : 65 * b + bp],
                        rhs=X2[:, b, 1, :],
                        start=False,
                        stop=(b == bp - 1),
                    )
                ad_eg = p2w.tile([bp, 1], F32, tag="ad_eg")
                nc.vector.tensor_scalar_mul(ad_eg, adust, eg)
                denw = p2w.tile([bp, NT], F32, tag="denw")
                nc.vector.tensor_scalar(
                    out=denw, in0=psT, scalar1=ad_eg, scalar2=None, op0=ALU.add
                )
                recw = p2w.tile([bp, NT], F32, tag="recw")
                nc.vector.reciprocal(out=recw, in_=denw)
                nc.vector.tensor_scalar_mul(bmat_rm, recw, NU_R)
                # b_dust = DUST / (eg * sum_n a_total)
                sa = p2w.tile([bp, 1], F32, tag="sa")
                nc.vector.tensor_reduce(
                    out=sa, in_=Amat, axis=mybir.AxisListType.X, op=ALU.add
                )
                sa2 = p2w.tile([bp, 1], F32, tag="sa2")
                nc.vector.tensor_scalar_mul(sa2, sa, eg)
                sa3 = p2w.tile([bp, 1], F32, tag="sa3")
                nc.vector.reciprocal(out=sa3, in_=sa2)
                nc.vector.tensor_scalar_mul(bdust, sa3, DUST)
                psBT = ppt.tile([128, bp], BF16, tag="psBT")
                nc.tensor.transpose(
                    out=psBT, in_=bmat_rm, identity=ident_bf[0:bp, 0:bp]
                )

            # -- loss = a^T M b per batch --
            nc.vector.tensor_copy(out=slots(Bdiag), in_=psBT)
            psL = pps.tile([bp, 256], F32, tag="psS")
            for b in range(bp):
                nc.tensor.matmul(
                    psL,
                    lhsT=Bdiag[:, 65 * b : 65 * b + bp],
                    rhs=M_all[:, b, :],
                    start=(b == 0),
                    stop=(b == bp - 1),
                )
            ltmp = p2w.tile([bp, 256], F32, tag="ltmp")
            nc.vector.tensor_mul(out=ltmp, in0=psL, in1=Amat[:, 0:NV])
            nc.vector.tensor_reduce(
                out=lossc, in_=ltmp, axis=mybir.AxisListType.X, op=ALU.add
            )
            nc.sync.dma_start(out=out[:, :], in_=lossc)


_nc_cache: dict = {}


def _numpy_fallback(v, t, v_mask, t_mask, gamma):
    """Exact numpy port of the reference (for non-all-ones masks)."""
    NEG_INF = -1e6
    v = v.astype(np.float32)
    t = t.astype(np.float32)
    vn = v / np.maximum(np.sqrt((v * v).sum(-1, keepdims=True)), 1e-12)
    tn = t / np.maximum(np.sqrt((t * t).sum(-1, keepdims=True)), 1e-12)
    A = np.einsum("bnd,bmd->bnm", vn, tn).astype(np.float32)
    A_raw = A.copy()
    A = np.where(v_mask[:, :, None], A, NEG_INF)
    A = np.where(t_mask[:, None, :], A, NEG_INF)
    Bn = A.shape[0]
    g = np.float32(gamma)
    A_aug = np.concatenate([A, np.full((Bn, NV, 1), g, np.float32)], axis=2)
    A_aug = np.concatenate(
        [A_aug, np.full((Bn, 1, NT + 1), g, np.float32)], axis=1
    )
    v_counts = v_mask.sum(1, keepdims=True) + 1e-9
    mu_real = v_mask.astype(np.float32) / v_counts
    t_counts = t_mask.sum(1, keepdims=True) + 1e-9
    nu_real = t_mask.astype(np.float32) / t_counts
    ones = np.ones((Bn, 1), np.float32)
    mu = np.concatenate([mu_real, ones], 1)
    nu = np.concatenate([nu_real, ones], 1)
    K = A_aug / EPS
    log_mu = np.log(mu + 1e-9)
    log_nu = np.log(nu + 1e-9)
    u = np.zeros_like(mu)
    w = np.zeros_like(nu)

    def lse(x, axis):
        m = x.max(axis=axis, keepdims=True)
        return (m + np.log(np.exp(x - m).sum(axis=axis, keepdims=True))).squeeze(axis)

    for _ in range(ITERS):
        u = log_mu - lse(K + w[:, None, :], 2)
        w = log_nu - lse(K + u[:, :, None], 1)
    T = np.exp(u[:, :, None] + w[:, None, :] + K)
    loss = (T[:, :NV, :NT] * (1.0 - A_raw)).sum((1, 2))
    return np.float32(loss.mean())


import os


def kernel(v, t, v_mask, t_mask, gamma):
    v = np.ascontiguousarray(np.asarray(v), dtype=np.float32)
    t = np.ascontiguousarray(np.asarray(t), dtype=np.float32)
    v_mask = np.asarray(v_mask)
    t_mask = np.asarray(t_mask)
    gamma_f = float(np.asarray(gamma))

    if not (v_mask.all() and t_mask.all()):
        return _numpy_fallback(v, t, v_mask, t_mask, gamma_f)

    try:
        eg = float(np.exp(np.float32(gamma_f) / np.float32(EPS)))
        key = (eg, v.shape, t.shape)
        if key not in _nc_cache:
            _nc_cache[key] = build_bass(eg)
        nc = _nc_cache[key]

        # host-side row norms (reference-exact f32 semantics)
        inv_v = 1.0 / np.maximum(
            np.sqrt((v * v).sum(-1)), np.float32(1e-12)
        ).astype(np.float32)
        inv_t = 1.0 / np.maximum(
            np.sqrt((t * t).sum(-1)), np.float32(1e-12)
        ).astype(np.float32)
        invs = np.empty((B, 128, 3), np.float32)
        invs[:, :, 0] = inv_v[:, 0:128]
        invs[:, :, 1] = inv_v[:, 128:256]
        invs[:, :, 2] = inv_t
        invs = np.ascontiguousarray(invs)

        in_maps = [
            {
                "v": v[i * BP : (i + 1) * BP],
                "t": t[i * BP : (i + 1) * BP],
                "invs": invs[i * BP : (i + 1) * BP],
            }
            for i in range(NCORES)
        ]
        res = run_bass_kernel_spmd(nc, in_maps, core_ids=list(range(NCORES)))
        losses = np.concatenate([r["out"][:, 0] for r in res.results])
        return np.float32(np.mean(losses.astype(np.float64)))
    except Exception:
        if os.environ.get("KERNEL_NO_FALLBACK"):
            raise
        return _numpy_fallback(v, t, v_mask, t_mask, gamma_f)


if __name__ == "__main__":
    rng = np.random.default_rng(0)
    v = rng.standard_normal((B, NV, D), dtype=np.float32)
    t = rng.standard_normal((B, NT, D), dtype=np.float32)
    vm = np.ones((B, NV), bool)
    tm = np.ones((B, NT), bool)
    print(kernel(v, t, vm, tm, np.float32(0.1)))
